# revision 4
# baseline (speedup 1.0000x reference)
"""Trainium2 Bass kernel for the CaptionDecoder problem (2-layer LSTM + vocab
projection) — transposed-gates formulation.

Sharding (8 NeuronCores): recurrence replicated, vocab-dim of the output
projection sharded 8-way (each core computes a [2016, 4000] logits slice).

Key idea vs the streaming baseline: compute gate pre-activations TRANSPOSED
(gates on partitions, batch on the free dim) with the weights as the matmul
stationary operand. Each [128x128] weight chunk then costs only N=32 moving
cycles instead of streaming 512 weight columns, giving ~4x PE efficiency.
h comes out of the elementwise chain already transposed ([h, batch] layout),
so the per-step PE transposes of the baseline disappear entirely, and the
elementwise work runs on all 128 partitions. Logits are written bf16 and
upconverted on host.

Numerics: matmuls bf16 with fp32 PSUM accumulation; activated gates bf16;
cell state fp32.
"""

import numpy as np
import ml_dtypes

import concourse.bass as bass
import concourse.mybir as mybir
import concourse.tile as tile
from concourse.vector_clock import ScopedClock
from concourse.bass_utils import run_bass_kernel_spmd

# ----------------------------------------------------------------------------
# Problem constants (hardcoded per harness contract)
# ----------------------------------------------------------------------------
B = 32          # batch
SEQ = 64        # caption length; recurrence runs on captions[:, :-1]
T = SEQ - 1     # 63 steps
E = 512         # embed dim
H = 512         # hidden dim
V = 32000       # vocab
NCORES = 8
VSH = V // NCORES   # 4000 vocab columns per core
TB = T * B          # 2016 (t-major token index: j = t*B + b)
NIDX = 2048         # token stream padded to a multiple of 128
GATES = 4 * H       # 2048 gates per layer ([f; i; o; c] blocks)
NG = GATES // 128   # 16 gate chunks
NK = 4              # 4 k-chunks of 128 for E or H
SLOTS = T + 1       # h-state slots (slot s = state entering step s)
NCH = 8             # vocab chunks of 500 columns
NM = (TB + 127) // 128  # 16 token chunks; last one is 96 rows

F32 = mybir.dt.float32
BF16 = mybir.dt.bfloat16
AF = mybir.ActivationFunctionType

bf16 = ml_dtypes.bfloat16


class SplitDrainTileContext(tile.TileContext):
    """TileContext whose tail drain splits its sem waits into single-wait
    instructions — the walrus build in this container accepts only one sync
    wait on a Drain."""

    def _drain_and_barrier(self, tick_clock, wait_clock):
        nc = self.nc
        drain_inst = nc.sync.drain()
        wait_clock.add_sem_waits(
            drain_inst.ins, ScopedClock({None: tick_clock.global_clock})
        )
        waits = list(drain_inst.ins.sync_info.on_wait or [])
        if len(waits) > 1:
            drain_inst.ins.sync_info.on_wait = [waits[0]]
            id2h = {h.num: h for h in wait_clock.sems.allocated().values()}
            for w in waits[1:]:
                assert w.wait_mode == "sem-ge-imm", w
                nc.sync.wait_ge(id2h[w.id], w.wait_value)

        nc.all_engine_barrier()
        assert self.sems is not None
        popped = nc._tile_sem_poison_stack.pop()
        assert popped is self._sem_poison
        nc.clear_and_free_semaphores(list(self.sems.allocated().values()))
        nc.all_engine_barrier()


def _split_excess_waits(nc, limit=1):
    """The walrus build in this container rejects instructions carrying more
    than one sync-wait command. Hoist excess waits onto standalone
    EventSemaphore instructions inserted just before the owner, on the same
    engine (conservative: the engine stalls where the queue would have)."""
    import bass_rust

    n_extra = 0
    for bb in nc.m.functions[0].blocks:
        insts = bb.instructions
        out = []
        for ins in insts:
            si = ins.sync_info
            waits = list(si.on_wait) if si and si.on_wait else []
            if len(waits) > limit:
                for w in waits[:-limit]:
                    n_extra += 1
                    wi = bass_rust.InstEventSemaphore(
                        name=f"WSPLIT-{n_extra}", ins=[], outs=[]
                    )
                    wi.engine = ins.engine
                    wi.sync_info = bass_rust.SyncInfo(on_wait=[w], on_update=[])
                    nc.register_instruction(wi)
                    out.append(wi)
                si.on_wait = waits[-limit:]
            out.append(ins)
        insts[:] = out
    return n_extra


# ----------------------------------------------------------------------------
# Device program
# ----------------------------------------------------------------------------

def _build_program(t_eff=T, do_proj=True, dbg=False):
    nc = bass.Bass("TRN2", target_bir_lowering=False, debug=False, num_devices=1)

    # -------- I/O --------
    embTd = nc.dram_tensor("embTd", [E, NIDX], BF16, kind="ExternalInput")
    featT = nc.dram_tensor("featT", [E, B], BF16, kind="ExternalInput")
    initw = nc.dram_tensor("initw", [E, 2 * H], BF16, kind="ExternalInput")
    initb = nc.dram_tensor("initb", [1, 2 * H], BF16, kind="ExternalInput")
    w0 = nc.dram_tensor("w0", [E + H, GATES], BF16, kind="ExternalInput")
    b0 = nc.dram_tensor("b0", [1, GATES], BF16, kind="ExternalInput")
    w1 = nc.dram_tensor("w1", [2 * H, GATES], BF16, kind="ExternalInput")
    b1 = nc.dram_tensor("b1", [1, GATES], BF16, kind="ExternalInput")
    outw = nc.dram_tensor("outw", [H, VSH], BF16, kind="ExternalInput")
    outb = nc.dram_tensor("outb", [128, VSH], BF16, kind="ExternalInput")
    ones = nc.dram_tensor("ones", [1, 128], BF16, kind="ExternalInput")
    logits = nc.dram_tensor("logits", [TB, VSH], BF16, kind="ExternalOutput")
    if dbg:
        h0Td = nc.dram_tensor("h0Td", [128, NK, SLOTS * B], BF16,
                              kind="ExternalOutput")
        h1Td = nc.dram_tensor("h1Td", [128, NK, SLOTS * B], BF16,
                              kind="ExternalOutput")

    with SplitDrainTileContext(nc) as tc:
        with tc.tile_pool(name="static", bufs=1) as wpool:
            # -------- static loads --------
            w0_t = wpool.tile([128, 2 * NK, GATES], BF16)
            nc.sync.dma_start(
                out=w0_t[:], in_=w0.ap().rearrange("(c p) n -> p c n", p=128)
            )
            b0_t = wpool.tile([1, GATES], BF16)
            nc.sync.dma_start(out=b0_t[:], in_=b0.ap())
            w1_t = wpool.tile([128, 2 * NK, GATES], BF16)
            nc.sync.dma_start(
                out=w1_t[:], in_=w1.ap().rearrange("(c p) n -> p c n", p=128)
            )
            b1_t = wpool.tile([1, GATES], BF16)
            nc.sync.dma_start(out=b1_t[:], in_=b1.ap())
            outw_t = wpool.tile([128, NK, VSH], BF16)
            nc.sync.dma_start(
                out=outw_t[:], in_=outw.ap().rearrange("(c p) n -> p c n", p=128)
            )
            outb_t = wpool.tile([128, VSH], BF16)
            nc.sync.dma_start(out=outb_t[:], in_=outb.ap())
            ones_t = wpool.tile([1, 128], BF16)
            nc.sync.dma_start(out=ones_t[:], in_=ones.ap())
            embT = wpool.tile([128, NK, NIDX], BF16)
            nc.sync.dma_start(
                out=embT[:], in_=embTd.ap().rearrange("(c p) n -> p c n", p=128)
            )

            # transposed state history: [128 (h-dim within chunk), k-chunk,
            # slot*B]; slot s = state entering step s
            h0T = wpool.tile([128, NK, SLOTS * B], BF16)
            h1T = wpool.tile([128, NK, SLOTS * B], BF16)

            with (
                tc.tile_pool(name="gst", bufs=3) as gpool,
                tc.tile_pool(name="tmp", bufs=3) as tpool,
                tc.tile_pool(name="cst", bufs=2) as cpool,
                tc.tile_pool(name="obnc", bufs=6) as opool,
                tc.tile_pool(name="ps0", bufs=2, space="PSUM") as ps0p,
                tc.tile_pool(name="ps1", bufs=2, space="PSUM") as ps1p,
                tc.tile_pool(name="ppsum", bufs=3, space="PSUM") as ppsum,
            ):
                # ---- init: [h0 | c0]^T = (features @ [init_h_w|init_c_w].T
                #            + b)^T via transposed matmuls ----
                c_state = [None, None]
                with (
                    tc.tile_pool(name="prolog", bufs=1) as ppool,
                    tc.tile_pool(name="ipsum", bufs=1, space="PSUM") as ipsum,
                ):
                    featT_t = ppool.tile([128, NK, B], BF16)
                    nc.sync.dma_start(
                        out=featT_t[:],
                        in_=featT.ap().rearrange("(c p) n -> p c n", p=128),
                    )
                    initw_t = ppool.tile([128, NK, 2 * H], BF16)
                    nc.sync.dma_start(
                        out=initw_t[:],
                        in_=initw.ap().rearrange("(c p) n -> p c n", p=128),
                    )
                    initb_t = ppool.tile([1, 2 * H], BF16)
                    nc.sync.dma_start(out=initb_t[:], in_=initb.ap())

                    ips = ipsum.tile([128, 8, B], F32)
                    for j in range(8):  # 4 h chunks then 4 c chunks
                        jc = slice(j * 128, (j + 1) * 128)
                        nc.tensor.matmul(
                            ips[:, j, :], initb_t[0:1, jc], ones_t[0:1, 0:B],
                            start=True, stop=False,
                        )
                        for k in range(NK):
                            nc.tensor.matmul(
                                ips[:, j, :], initw_t[:, k, jc], featT_t[:, k, :],
                                start=False, stop=(k == NK - 1),
                            )
                    # h^T chunks -> slot 0 of both layers (bf16)
                    nc.vector.tensor_copy(h0T[:, :, 0:B], ips[:, 0:4, :])
                    nc.vector.tensor_copy(h1T[:, :, 0:B], ips[:, 0:4, :])
                    # c^T chunks -> per-layer state (f32)
                    for l in range(2):
                        ct = cpool.tile([128, 4, B], F32, tag=f"c{l}")
                        nc.vector.tensor_copy(ct[:], ips[:, 4:8, :])
                        c_state[l] = ct

                def project_unit(m, n):
                    cs = min(128, TB - m * 128)
                    ms = slice(B + m * 128, B + m * 128 + cs)  # slots 1..63
                    ns = slice(n * 500, (n + 1) * 500)
                    pps = ppsum.tile([128, 500], F32, tag="pp")
                    for k in range(NK):
                        nc.tensor.matmul(
                            pps[:cs, :], h1T[:, k, ms], outw_t[:, k, ns],
                            start=(k == 0), stop=(k == NK - 1),
                        )
                    osb = opool.tile([128, 500], BF16, tag="ob")
                    nc.vector.tensor_add(osb[:cs, :], pps[:cs, :],
                                         outb_t[:cs, ns])
                    nc.sync.dma_start(
                        out=logits.ap()[m * 128 : m * 128 + cs, ns],
                        in_=osb[:cs, :],
                    )

                def project_chunk(m):
                    for n in range(NCH):
                        project_unit(m, n)

                def lstm_layer(l, t, w_t, b_t, xT, hT):
                    """Emit one transposed LSTM cell step: gates from x-part
                    (xT slot t+1 for l=1, slot t for l=0) and h-part (hT slot
                    t), writing the new h into hT slot t+1."""
                    x_sl = slice((t + l) * B, (t + l + 1) * B)
                    h_in = slice(t * B, (t + 1) * B)
                    h_out = slice((t + 1) * B, (t + 2) * B)
                    psp = ps0p if l == 0 else ps1p
                    ps = psp.tile([128, NG, B], F32, tag=f"g{l}")
                    for g in range(NG):
                        gc = slice(g * 128, (g + 1) * 128)
                        nc.tensor.matmul(
                            ps[:, g, :], b_t[0:1, gc], ones_t[0:1, 0:B],
                            start=True, stop=False,
                        )
                        for k in range(NK):
                            nc.tensor.matmul(
                                ps[:, g, :], w_t[:, k, gc], xT[:, k, x_sl],
                                start=False, stop=False,
                            )
                        for k in range(NK):
                            nc.tensor.matmul(
                                ps[:, g, :], w_t[:, NK + k, gc], hT[:, k, h_in],
                                start=False, stop=(k == NK - 1),
                            )
                    # activations: f,i,o sigmoid (chunks 0..11), c~ tanh
                    gs = gpool.tile([128, NG, B], BF16, tag=f"gs{l}")
                    nc.scalar.activation(gs[:, 0:12, :], ps[:, 0:12, :],
                                         AF.Sigmoid)
                    nc.scalar.activation(gs[:, 12:16, :], ps[:, 12:16, :],
                                         AF.Tanh)
                    # cell update (all [128, 4, B] = 4 h-chunks x batch)
                    t1 = tpool.tile([128, 4, B], F32, tag=f"t1{l}")
                    nc.vector.tensor_mul(t1[:], gs[:, 0:4, :], c_state[l][:])
                    t2 = tpool.tile([128, 4, B], F32, tag=f"t2{l}")
                    nc.gpsimd.tensor_mul(t2[:], gs[:, 4:8, :], gs[:, 12:16, :])
                    c_new = cpool.tile([128, 4, B], F32, tag=f"c{l}")
                    nc.gpsimd.tensor_add(c_new[:], t1[:], t2[:])
                    c_state[l] = c_new
                    tch = tpool.tile([128, 4, B], BF16, tag=f"tc{l}")
                    nc.scalar.activation(tch[:], c_new[:], AF.Tanh)
                    # h_new = o * tanh(c), written transposed into slot t+1
                    nc.vector.tensor_mul(hT[:, :, h_out], gs[:, 8:12, :],
                                         tch[:])

                # ---- software-pipelined recurrence with interleaved
                # projection. Token chunk m reads h1T slots 4m+1..4m+4
                # (written by L1 steps up to 4m+3, i.e. by iteration 4m+4),
                # so its (m, n) units unlock at iteration 4m+5. Emit ~2 units
                # per iteration to smooth PE occupancy — fill work while the
                # serial elementwise chain of the current step completes.
                units = [(m, n) for m in range(NM) for n in range(NCH)]
                emitted = 0
                for i in range(t_eff + 1):
                    if do_proj:
                        while (emitted < len(units)
                               and emitted < 2 * max(0, i - 4)
                               and units[emitted][0] * 4 + 5 <= i):
                            project_unit(*units[emitted])
                            emitted += 1
                    if i < t_eff:
                        lstm_layer(0, i, w0_t, b0_t, embT, h0T)
                    if i >= 1:
                        lstm_layer(1, i - 1, w1_t, b1_t, h0T, h1T)

                # tail units
                for m, n in units[emitted:]:
                    project_unit(m, n)

                if dbg:
                    nc.sync.dma_start(out=h0Td.ap(), in_=h0T[:])
                    nc.sync.dma_start(out=h1Td.ap(), in_=h1T[:])

    _split_excess_waits(nc)
    return nc


_NC_CACHE = None


def _get_program():
    global _NC_CACHE
    if _NC_CACHE is None:
        _NC_CACHE = _build_program()
    return _NC_CACHE


# ----------------------------------------------------------------------------
# Host-side input prep / output gather
# ----------------------------------------------------------------------------

def _prepare_in_maps(inputs):
    bf = lambda a: np.ascontiguousarray(np.asarray(a, dtype=np.float32).astype(bf16))

    cap = np.asarray(inputs["captions"])[:, :-1]  # [B, T]
    tbl = bf(inputs["embedding_w"])
    embT = np.zeros((E, NIDX), bf16)
    embT[:, :TB] = tbl[cap.T.reshape(-1)].T
    featT = bf(np.asarray(inputs["features"]).T)
    initw = bf(np.concatenate(
        [np.asarray(inputs["init_h_w"]).T, np.asarray(inputs["init_c_w"]).T], axis=1))
    initb = bf(np.concatenate(
        [np.asarray(inputs["init_h_b"]), np.asarray(inputs["init_c_b"])])[None, :])

    wl, bl = [], []
    for l in range(2):
        W = np.concatenate(
            [np.asarray(inputs[f"W{g}"])[l] for g in "fioc"], axis=0)  # [2048, 1024]
        bias = np.concatenate([np.asarray(inputs[f"b{g}"])[l] for g in "fioc"])
        wl.append(bf(W.T))
        bl.append(bf(bias[None, :]))

    ones = np.ones((1, 128), bf16)

    out_w = np.asarray(inputs["out_w"])
    out_b = np.asarray(inputs["out_b"])

    in_maps = []
    for c in range(NCORES):
        vs = slice(c * VSH, (c + 1) * VSH)
        in_maps.append({
            "embTd": embT,
            "featT": featT,
            "initw": initw,
            "initb": initb,
            "w0": wl[0], "b0": bl[0],
            "w1": wl[1], "b1": bl[1],
            "outw": bf(out_w[vs].T),
            "outb": np.ascontiguousarray(
                np.broadcast_to(out_b[vs].astype(bf16)[None, :], (128, VSH))),
            "ones": ones,
        })
    return in_maps


def _run(inputs, trace=False):
    nc = _get_program()
    in_maps = _prepare_in_maps(inputs)
    res = run_bass_kernel_spmd(
        nc, in_maps, core_ids=list(range(NCORES)), trace=trace
    )
    shards = [
        np.asarray(res.results[c]["logits"], dtype=np.float32).reshape(T, B, VSH)
        for c in range(NCORES)
    ]
    full = np.concatenate(shards, axis=2).swapaxes(0, 1)  # [B, T, V]
    return np.ascontiguousarray(full, dtype=np.float32), res


def kernel(**inputs) -> np.ndarray:
    out, _ = _run(inputs, trace=False)
    return out


def kernel_with_stats(**inputs):
    out, res = _run(inputs, trace=True)
    return out, res


def _build_null_program():
    """Trivial 8-core kernel used to measure dispatch overhead."""
    nc = bass.Bass("TRN2", target_bir_lowering=False, debug=False, num_devices=1)
    x = nc.dram_tensor("x", [128, 128], F32, kind="ExternalInput")
    y = nc.dram_tensor("y", [128, 128], F32, kind="ExternalOutput")
    with SplitDrainTileContext(nc) as tc:
        with tc.tile_pool(name="sbuf", bufs=1) as pool:
            t = pool.tile([128, 128], F32)
            nc.sync.dma_start(out=t[:], in_=x.ap())
            nc.sync.dma_start(out=y.ap(), in_=t[:])
    _split_excess_waits(nc)
    return nc


def _make_runner(nc, in_maps):
    """Build a jitted 8-core runner for `nc` with device-resident inputs
    (no donation, results left on device); returns a zero-arg callable."""
    import jax
    from jax.sharding import Mesh, PartitionSpec, NamedSharding
    from jax.experimental.shard_map import shard_map
    from concourse.bass2jax import (
        _bass_exec_p, install_neuronx_cc_hook, partition_id_tensor,
    )

    install_neuronx_cc_hook()
    partition_name = (
        nc.partition_id_tensor.name if nc.partition_id_tensor else None
    )
    in_names, out_names, out_avals, zero_outs = [], [], [], []
    for alloc in nc.m.functions[0].allocations:
        if not isinstance(alloc, mybir.MemoryLocationSet):
            continue
        name = alloc.memorylocations[0].name
        if alloc.kind == "ExternalInput":
            if name != partition_name:
                in_names.append(name)
        elif alloc.kind == "ExternalOutput":
            out_names.append(name)
            shape = tuple(alloc.tensor_shape)
            dtype = mybir.dt.np(alloc.dtype)
            out_avals.append(jax.core.ShapedArray(shape, dtype))
            zero_outs.append(np.zeros(shape, dtype))
    n_params = len(in_names)
    n_outs = len(out_names)
    in_names_full = list(in_names) + out_names
    if partition_name:
        in_names_full.append(partition_name)

    def _body(*args):
        operands = list(args)
        if partition_name:
            operands.append(partition_id_tensor())
        outs = _bass_exec_p.bind(
            *operands,
            out_avals=tuple(out_avals),
            in_names=tuple(in_names_full),
            out_names=tuple(out_names),
            lowering_input_output_aliases=(),
            sim_require_finite=True,
            sim_require_nnan=True,
            nc=nc,
        )
        return tuple(outs)

    devices = jax.devices()[:NCORES]
    mesh = Mesh(np.asarray(devices), ("core",))
    spec = NamedSharding(mesh, PartitionSpec("core"))
    concat_in = [
        np.concatenate([np.asarray(in_maps[c][nm]) for c in range(NCORES)], axis=0)
        for nm in in_names
    ]
    concat_zeros = [
        np.zeros((NCORES * z.shape[0], *z.shape[1:]), z.dtype) for z in zero_outs
    ]
    dev_in = [jax.device_put(a, spec) for a in concat_in]
    dev_zero = [jax.device_put(a, spec) for a in concat_zeros]

    fn = jax.jit(shard_map(
        _body, mesh=mesh,
        in_specs=(PartitionSpec("core"),) * (n_params + n_outs),
        out_specs=(PartitionSpec("core"),) * n_outs,
        check_rep=False,
    ))

    def run():
        r = fn(*dev_in, *dev_zero)
        jax.block_until_ready(r)

    return run


def _timed_runner(nc, in_maps, iters):
    import time

    run = _make_runner(nc, in_maps)
    run()  # compile + warm
    best = None
    for _ in range(iters):
        t0 = time.perf_counter_ns()
        run()
        dt = time.perf_counter_ns() - t0
        best = dt if best is None else min(best, dt)
    return best


def benchmark(inputs, iters=40):
    """Estimate device execution time of the kernel: min wall time of the
    full kernel minus min wall time of a trivial kernel (same dispatch
    path). Full/null samples are interleaved so tunnel-latency drift
    affects both distributions equally. Returns (per_exec_ns, details)."""
    import time

    run_full = _make_runner(_get_program(), _prepare_in_maps(inputs))
    nc_null = _build_null_program()
    null_maps = [{"x": np.zeros((128, 128), np.float32)} for _ in range(NCORES)]
    run_null = _make_runner(nc_null, null_maps)

    run_full()
    run_null()  # compile + warm both
    t_full, t_null = [], []
    for _ in range(iters):
        t0 = time.perf_counter_ns()
        run_full()
        t_full.append(time.perf_counter_ns() - t0)
        t0 = time.perf_counter_ns()
        run_null()
        t_null.append(time.perf_counter_ns() - t0)
    mf, mn = min(t_full), min(t_null)
    return mf - mn, {"full": mf, "null": mn}
